# revision 26
# baseline (speedup 1.0000x reference)
"""Trainium2 Bass kernel for nn_HA_unit (gnn_message_passing).

Math (per batch b, N = H*W spatial positions):
  wfeat = BN1(w1 @ x)                       [IC, N]   (BN folded on host)
  iw    = wfeat^T wfeat * IC^-0.5           [N, N]    symmetric
  nodes = node_w @ x + node_b               [IC, N]   (kept as [N, IC])
  b0    = (sigmoid(iw) >= delta)            [N, N]    binary, symmetric
  bh_k  = b0^k  (k = 1, 2, 3)               exact integer counts
  hop_k = hopw_k @ (softmax(bh_k o iw) @ nodes)^T + hopb_k
  xp    = BNf(fuse_w @ concat(hops))
  out   = BNr(res_w @ concat(x[:IC], xp))

Sharding: 8 cores = 4 batches x 2 halves of N. Core (b, h) receives x[b]
with spatial positions rolled by h*N/2 so that its rows are always 0..N/2-1
(identical SPMD program, data-only difference). Each core computes the full
symmetric b0 locally (no collectives); the b0^2 / b0^3 matmuls are sharded
by output rows. Binary/int matmul operands are exact in bf16/f32.

Perf structure:
  - b0^2 (phase C) runs fp8 DoubleRow (2 K-planes per matmul).
  - bh2^T for phase D is produced with xbar DMA transposes, not PE.
  - DMA issue queues are split: streaming loads go through SWDGE
    (gpsimd), xbar transposes through ACT's HWDGE ring, stores through
    SP's — the SP ring alone saturates otherwise.
  - phase D's PSUM eviction is fused with the elementwise iw product, so
    it writes hop-2 softmax logits directly (no bh3 round trip).
  - softmax work is emitted interleaved with D's matmul blocks: the PE
    stream stays dense while DVE/ACT chew the softmax chains.
  - hop0 logits are bounded (|iw| <~ 6) so its max-reduce is skipped.
"""

import sys
from contextlib import ExitStack

sys.path.insert(0, "/opt/trn_rl_repo")

import numpy as np

P = 128


def _build(cin, ic, n, r, hop, thr):
    from concourse import bass, tile, bacc
    import concourse.mybir as mybir

    f32 = mybir.dt.float32
    f16 = mybir.dt.float16
    fp8 = mybir.dt.float8e4
    AF = mybir.ActivationFunctionType
    ALU = mybir.AluOpType
    AX = mybir.AxisListType
    DR = mybir.MatmulPerfMode.DoubleRow

    ncin = cin // P          # K-chunks over input channels
    nic = ic // P            # chunks over inter channels
    nkn = n // P             # K-chunks over N
    nrt = r // P             # our row tiles
    FB = min(512, n)         # free-dim blocking
    nfb = n // FB
    hc = hop * ic
    nhc = hc // P
    cout = cin
    ncout = cout // P
    nxc = ic // P            # x residual slice chunks (x[:ic])

    SB = 2                   # row tiles per softmax super-block
    nsb = nrt // SB          # super-blocks (8)
    RH = min(512, r)         # D row-block
    nrh = r // RH            # 4
    nrq = RH // P            # 4
    spb = nsb // nrh         # E super-blocks per D block (2)

    # bias_pack columns: [b1(nic) | hop(hop*nic) | fuse(nic) | res(ncout)]
    C_B1 = 0
    C_HOP = C_B1 + nic
    C_FUSE = C_HOP + hop * nic
    C_RES = C_FUSE + nic
    NBIAS = C_RES + ncout

    nc = bacc.Bacc("TRN2", target_bir_lowering=False, debug=False)

    x16 = nc.dram_tensor("x16", [cin, n], f16, kind="ExternalInput")
    w1T = nc.dram_tensor("w1T", [cin, ic], f16, kind="ExternalInput")
    nodeT = nc.dram_tensor("nodeT", [cin, ic], f16, kind="ExternalInput")
    nbrow = nc.dram_tensor("nbrow", [1, ic], f16, kind="ExternalInput")
    hopT = nc.dram_tensor("hopT", [hop, ic, ic], f16, kind="ExternalInput")
    fuseT = nc.dram_tensor("fuseT", [hc, ic], f16, kind="ExternalInput")
    resT = nc.dram_tensor("resT", [2 * ic, cout], f16, kind="ExternalInput")
    biases = nc.dram_tensor("biases", [P, NBIAS], f32, kind="ExternalInput")
    out = nc.dram_tensor("out", [cout, r], f32, kind="ExternalOutput")

    with tile.TileContext(nc) as tc:
        with (
            tc.tile_pool(name="dram", bufs=1, space="DRAM") as dpool,
            tc.tile_pool(name="consts", bufs=1) as consts,
            tc.tile_pool(name="pf_const", bufs=1) as pfc,
        ):
            BND = n // 4
            b0qb = [
                dpool.tile([BND, n], fp8, tag=f"b0q{j}", name=f"b0q{j}")
                for j in range(4)
            ]
            kpb = BND // P  # k-chunks per band (8)
            iwq = dpool.tile([r, n], f32, tag="iwq")
            bh2h = dpool.tile([r, n], f16, tag="bh2h")
            bh2T = dpool.tile([n, r], f16, tag="bh2T")
            lg3d = dpool.tile([r, n], f32, tag="lg3d")
            nodes_d = dpool.tile([n, ic], f16, tag="nodes_d")
            hops_d = [
                dpool.tile([ic, r], f16, tag=f"hops_d{i}", name=f"hops_d{i}")
                for i in range(hop)
            ]

            bias_sb = consts.tile([P, NBIAS], f32, tag="bias_sb")
            nc.sync.dma_start(bias_sb[:], biases[:])
            ones1 = consts.tile([1, P], f16, tag="ones1")
            nc.vector.memset(ones1[:], 1.0)
            nbrow_sb = consts.tile([1, ic], f16, tag="nbrow_sb")
            nc.sync.dma_start(nbrow_sb[:], nbrow[:])

            # ---------------- Phase A: wfeat + nodes ----------------
            with (
                tc.tile_pool(name="pa", bufs=1) as pa,
                tc.tile_pool(name="psA", bufs=2, space="PSUM") as psA,
                tc.tile_pool(name="evA", bufs=2) as evict,
                tc.tile_pool(name="rowA", bufs=2) as rowp,
            ):
                x_sb = pa.tile([P, ncin, n], f16, tag="x_sb")
                for k in range(ncin):
                    nc.gpsimd.dma_start(
                        x_sb[:, k, :],
                        x16[k * P:(k + 1) * P, :],
                    )
                w1T_sb = pa.tile([P, ncin, ic], f16, tag="w1T_sb")
                nc.gpsimd.dma_start(
                    w1T_sb[:], w1T[:, :].rearrange("(k p) o -> p k o", p=P)
                )
                nodeT_sb = pa.tile([P, ncin, ic], f16, tag="nodeT_sb")
                nc.gpsimd.dma_start(
                    nodeT_sb[:], nodeT[:, :].rearrange("(k p) o -> p k o", p=P)
                )
                wf_sb = pa.tile([P, nic, n], f16, tag="wf_sb")

                for oc in range(nic):
                    for f in range(nfb):
                        ps = psA.tile([P, FB], f32, tag=f"ps{f % 2}")
                        for k in range(ncin):
                            nc.tensor.matmul(
                                ps[:],
                                w1T_sb[:, k, oc * P:(oc + 1) * P],
                                x_sb[:, k, f * FB:(f + 1) * FB],
                                start=(k == 0),
                                stop=(k == ncin - 1),
                            )
                        nc.scalar.activation(
                            wf_sb[:, oc, f * FB:(f + 1) * FB],
                            ps[:],
                            AF.Identity,
                            bias=bias_sb[:, C_B1 + oc:C_B1 + oc + 1],
                        )

                for nt in range(nkn):
                    ps = psA.tile([P, ic], f32, tag="psn")
                    for k in range(ncin):
                        nc.tensor.matmul(
                            ps[:],
                            x_sb[:, k, nt * P:(nt + 1) * P],
                            nodeT_sb[:, k, :],
                            start=(k == 0),
                            stop=False,
                        )
                    nc.tensor.matmul(
                        ps[:], ones1[:], nbrow_sb[:], start=False, stop=True
                    )
                    ev = evict.tile([P, ic], f16, tag="evn")
                    nc.vector.tensor_copy(ev[:], ps[:])
                    nc.sync.dma_start(nodes_d[nt * P:(nt + 1) * P, :], ev[:])

                # ---------------- Phase B: iw + b0 ----------------
                # evictions accumulate into full row blocks, one store per
                # row block, to keep the SP DMA ring shallow.
                for pc in range(nkn):
                    b0row = rowp.tile([P, n], fp8, tag="b0row")
                    iwrow = None
                    if pc * P < r:
                        iwrow = rowp.tile([P, n], f32, tag="iwrow")
                    for f in range(nfb):
                        ps = psA.tile([P, FB], f32, tag=f"ps{f % 2}")
                        for k in range(nic):
                            nc.tensor.matmul(
                                ps[:],
                                wf_sb[:, k, pc * P:(pc + 1) * P],
                                wf_sb[:, k, f * FB:(f + 1) * FB],
                                start=(k == 0),
                                stop=(k == nic - 1),
                            )
                        nc.vector.tensor_scalar(
                            b0row[:, f * FB:(f + 1) * FB], ps[:], thr, None,
                            op0=ALU.is_ge,
                        )
                        if iwrow is not None:
                            nc.scalar.activation(
                                iwrow[:, f * FB:(f + 1) * FB], ps[:], AF.Copy
                            )
                    nc.sync.dma_start(
                        b0qb[pc // kpb][(pc % kpb) * P:(pc % kpb + 1) * P, :],
                        b0row[:],
                    )
                    if iwrow is not None:
                        nc.sync.dma_start(
                            iwq[pc * P:(pc + 1) * P, :], iwrow[:]
                        )

            # ------- Phase C: bh2 = b0 @ b0 (fp8 DoubleRow, exact f32) -------
            with (
                tc.tile_pool(name="pc", bufs=1) as pcp,
                tc.tile_pool(name="pc_rhs", bufs=2) as pcr,
                tc.tile_pool(name="psC", bufs=2, space="PSUM") as psC,
                tc.tile_pool(name="evCp", bufs=2) as evictC,
            ):
                lh = pcp.tile([P, nkn, r], fp8, tag="lh")
                for j in range(4):
                    nc.gpsimd.dma_start(
                        lh[:, j * kpb:(j + 1) * kpb, :],
                        b0qb[j][:, 0:r].rearrange("(k p) q -> p k q", p=P),
                    )
                for mc in range(nfb):
                    rt_ = pcr.tile([P, nkn, FB], fp8, tag="rhsC")
                    for j in range(4):
                        nc.gpsimd.dma_start(
                            rt_[:, j * kpb:(j + 1) * kpb, :],
                            b0qb[j][:, mc * FB:(mc + 1) * FB].rearrange(
                                "(k p) q -> p k q", p=P
                            ),
                        )
                    for rq in range(nrt):
                        ps = psC.tile(
                            [P, FB], f32, tag=f"psC{rq % 4}", bufs=1
                        )
                        for k in range(0, nkn, 2):
                            nc.tensor.matmul(
                                ps[:],
                                lh[:, k:k + 2, rq * P:(rq + 1) * P],
                                rt_[:, k:k + 2, :],
                                start=(k == 0),
                                stop=(k == nkn - 2),
                                perf_mode=DR,
                            )
                        ev = evictC.tile(
                            [P, FB], f16, tag=f"evC{rq % 4}", bufs=1
                        )
                        nc.vector.tensor_copy(ev[:], ps[:])
                        rg = rq * P
                        nc.sync.dma_start(
                            bh2h[rg:rg + P, mc * FB:(mc + 1) * FB], ev[:]
                        )
                        if rq % 4 == 0:
                            tTb = evictC.tile(
                                [P, FB // P, 4 * P], f16,
                                tag=f"tT{(rq // 4) % 2}", bufs=1,
                            )
                        nc.scalar.dma_start(
                            tTb[:, :, (rq % 4) * P:(rq % 4 + 1) * P],
                            ev[:],
                            transpose=True,
                        )
                        if rq % 4 == 3:
                            # one contiguous 1KB-per-row store per 4 row
                            # tiles -- per-tile stores are 256B-descriptor
                            # scatters that clog the SP DMA ring
                            nc.sync.dma_start(
                                bh2T[
                                    mc * FB:(mc + 1) * FB, rg - 3 * P:rg + P
                                ].rearrange("(j p) q -> p j q", p=P),
                                tTb[:],
                            )

            # ---- Phases D (bh3 logits) + E (softmax hops), interleaved ----
            with ExitStack() as stk:
                ec = stk.enter_context
                pec = ec(tc.tile_pool(name="pe_const", bufs=1))
                peiw = ec(tc.tile_pool(name="pe_iw", bufs=1))
                pebh = ec(tc.tile_pool(name="pe_bh", bufs=1))
                pelg = ec(tc.tile_pool(name="pe_lg", bufs=1))
                pept = ec(tc.tile_pool(name="pe_pt", bufs=2))
                pesT = ec(tc.tile_pool(name="pe_sT", bufs=2))
                peo = ec(tc.tile_pool(name="pe_out", bufs=2))
                psE = ec(tc.tile_pool(name="psE", bufs=2, space="PSUM"))
                psH = ec(tc.tile_pool(name="psH", bufs=2, space="PSUM"))
                pdl = ec(tc.tile_pool(name="pd_lhs", bufs=1))
                pdr = ec(tc.tile_pool(name="pd_rhs", bufs=2))
                pdiw = ec(tc.tile_pool(name="pd_iw", bufs=2))
                psD = ec(tc.tile_pool(name="psD", bufs=2, space="PSUM"))
                evictD = ec(tc.tile_pool(name="evDp", bufs=2))
                fuseT_sb = pfc.tile([P, nhc, ic], f16, tag="fuseT_sb")
                nc.gpsimd.dma_start(
                    fuseT_sb[:], fuseT[:, :].rearrange("(k p) o -> p k o", p=P)
                )
                resT_sb = pfc.tile([P, 2 * nic, cout], f16, tag="resT_sb")
                nc.gpsimd.dma_start(
                    resT_sb[:], resT[:, :].rearrange("(k p) o -> p k o", p=P)
                )
                nodes_sb = pec.tile([P, nkn, ic], f16, tag="nodes_sb")
                nc.gpsimd.dma_start(
                    nodes_sb[:], nodes_d[:, :].rearrange("(t p) c -> p t c", p=P)
                )
                hopT_sb = pec.tile([P, hop, nic, ic], f16, tag="hopT_sb")
                for i in range(hop):
                    nc.gpsimd.dma_start(
                        hopT_sb[:, i, :, :],
                        hopT[i, :, :].rearrange("(k p) o -> p k o", p=P),
                    )

                def emit_chain(i, rt, sT, sl, iwt=None):
                    """Softmax chain for hop i, row tile rt; writes the
                    transposed, normalized probabilities into
                    sT[:, :, sl*P:(sl+1)*P]. Hops 0/1 share one iw row
                    load; hop0 uses ACT relu when thr == 0 (b0 o iw ==
                    relu(iw)) so no b0 row load or DVE mul is needed."""
                    if i == 2:
                        lg = pelg.tile([P, n], f32, tag="lg")
                        nc.gpsimd.dma_start(
                            lg[:], lg3d[rt * P:(rt + 1) * P, :]
                        )
                    elif i == 0 and thr == 0.0:
                        lg = pelg.tile([P, n], f32, tag="lg")
                        nc.scalar.activation(lg[:], iwt[:], AF.Relu)
                    else:
                        if i == 0:
                            bht = pebh.tile([P, n], fp8, tag="bh")
                            nc.gpsimd.dma_start(
                                bht[:],
                                b0qb[rt // kpb][
                                    (rt % kpb) * P:(rt % kpb + 1) * P, :
                                ],
                            )
                        else:
                            bht = pebh.tile([P, n], f16, tag="bh")
                            nc.gpsimd.dma_start(
                                bht[:], bh2h[rt * P:(rt + 1) * P, :]
                            )
                        lg = pelg.tile([P, n], f32, tag="lg")
                        nc.vector.tensor_mul(lg[:], iwt[:], bht[:])
                    nmax = None
                    if i > 0:
                        nmax = peo.tile([P, 1], f32, tag="nmax")
                        nc.vector.tensor_reduce(
                            nmax[:], lg[:], axis=AX.X, op=ALU.max, negate=True
                        )
                    zt = peo.tile([P, 1], f32, tag="zt")
                    pt = pept.tile([P, n], f16, tag="pt")
                    if nmax is not None:
                        nc.scalar.activation(
                            pt[:], lg[:], AF.Exp, bias=nmax[:], accum_out=zt[:]
                        )
                    else:
                        nc.scalar.activation(
                            pt[:], lg[:], AF.Exp, accum_out=zt[:]
                        )
                    rz = peo.tile([P, 1], f32, tag="rz")
                    nc.vector.reciprocal(rz[:], zt[:])
                    st = pept.tile([P, n], f16, tag="pt")
                    nc.scalar.activation(st[:], pt[:], AF.Copy, scale=rz[:])
                    nc.scalar.dma_start(
                        sT[:, :, sl * P:(sl + 1) * P], st[:], transpose=True
                    )

                def emit_chains(i, s):
                    """Emit the softmax chains for a todo entry. i == 01
                    emits hops 0 and 1 together, sharing one iw row load
                    per row tile; returns the super descriptors."""
                    if i == 1:
                        sT0 = pesT.tile([P, nkn, SB * P], f16, tag="sT")
                        sT1 = pesT.tile([P, nkn, SB * P], f16, tag="sT")
                        for sl in range(SB):
                            rt = s * SB + sl
                            iwt = peiw.tile([P, n], f32, tag="iwt")
                            nc.gpsimd.dma_start(
                                iwt[:], iwq[rt * P:(rt + 1) * P, :]
                            )
                            emit_chain(0, rt, sT0, sl, iwt)
                            emit_chain(1, rt, sT1, sl, iwt)
                        return [(0, s, sT0), (1, s, sT1)]
                    sT = pesT.tile([P, nkn, SB * P], f16, tag="sT")
                    for sl in range(SB):
                        emit_chain(i, s * SB + sl, sT, sl)
                    return [(i, s, sT)]

                def emit_super_mms(i, s, sT):
                    """t = s @ nodes and the hop conv for super-block s of
                    hop i (SB row tiles at once, FD=SB*P). Evictions stay
                    off ACT so the in-order ACT queue only carries the
                    softmax chains."""
                    W = SB * P
                    tcb = peo.tile([P, nic, W], f16, tag=f"tcb{s % 2}")
                    for c in range(nic):
                        ps = psE.tile([P, W], f32, tag="psE")
                        for j in range(nkn):
                            nc.tensor.matmul(
                                ps[:],
                                nodes_sb[:, j, c * P:(c + 1) * P],
                                sT[:, j, :],
                                start=(j == 0),
                                stop=(j == nkn - 1),
                            )
                        nc.scalar.activation(tcb[:, c, :], ps[:], AF.Copy)
                    hv = peo.tile([P, nic, W], f16, tag=f"hv{s % 2}")
                    for o in range(nic):
                        ph = psH.tile([P, W], f32, tag="psH")
                        for c in range(nic):
                            nc.tensor.matmul(
                                ph[:],
                                hopT_sb[:, i, c, o * P:(o + 1) * P],
                                tcb[:, c, :],
                                start=(c == 0),
                                stop=(c == nic - 1),
                            )
                        nc.scalar.activation(
                            hv[:, o, :],
                            ph[:],
                            AF.Identity,
                            bias=bias_sb[
                                :, C_HOP + i * nic + o:C_HOP + i * nic + o + 1
                            ],
                        )
                    nc.sync.dma_start(
                        hops_d[i][:, s * W:(s + 1) * W].rearrange(
                            "(o p) q -> p o q", p=P
                        ),
                        hv[:],
                    )

                # Interleave schedule per D block b (nfb=8 slots):
                #   slot m: emit chains for pending hop-super m (if any),
                #           then D's mc=m matmul group,
                #           then consume super m-offset's matmuls.
                # Chains get >=2 D-groups (~50us) of DVE/ACT headroom
                # before their PE matmuls, and every engine queue stays
                # aligned with consumption order.
                for b in range(nrh):
                    lh = pdl.tile([P, nkn, RH], f16, tag="lhD")
                    nc.gpsimd.dma_start(
                        lh[:],
                        bh2T[:, b * RH:(b + 1) * RH].rearrange(
                            "(k p) q -> p k q", p=P
                        ),
                    )
                    todo = []
                    for sp in range(spb):
                        todo.append((1, b * spb + sp))
                    if b > 0:
                        for sp in range(spb):
                            todo.append((2, (b - 1) * spb + sp))
                    nsup = 2 * spb + (spb if b > 0 else 0)
                    offset = nfb - nsup
                    filled = []
                    ncons = 0
                    for mc in range(nfb):
                        if mc >= offset and ncons < len(filled):
                            emit_super_mms(*filled[ncons])
                            ncons += 1
                        rt_ = pdr.tile([P, nkn, FB], fp8, tag="rhsD")
                        for j in range(4):
                            nc.gpsimd.dma_start(
                                rt_[:, j * kpb:(j + 1) * kpb, :],
                                b0qb[j][:, mc * FB:(mc + 1) * FB].rearrange(
                                    "(k p) q -> p k q", p=P
                                ),
                            )
                        for rq in range(nrq):
                            ps = psD.tile(
                                [P, FB], f32, tag=f"psD{rq % 4}", bufs=1
                            )
                            for k in range(nkn):
                                nc.tensor.matmul(
                                    ps[:],
                                    lh[:, k, rq * P:(rq + 1) * P],
                                    rt_[:, k, :],
                                    start=(k == 0),
                                    stop=(k == nkn - 1),
                                )
                            rg = b * RH + rq * P
                            iwd = pdiw.tile(
                                [P, FB], f32, tag=f"iwd{rq % 4}", bufs=1
                            )
                            nc.gpsimd.dma_start(
                                iwd[:],
                                iwq[rg:rg + P, mc * FB:(mc + 1) * FB],
                            )
                            ev = evictD.tile(
                                [P, FB], f32, tag=f"evD{rq % 4}", bufs=1
                            )
                            nc.vector.tensor_mul(ev[:], ps[:], iwd[:])
                            nc.sync.dma_start(
                                lg3d[rg:rg + P, mc * FB:(mc + 1) * FB], ev[:]
                            )
                        if mc < len(todo):
                            filled.extend(emit_chains(*todo[mc]))
                    for k in range(ncons, len(filled)):
                        emit_super_mms(*filled[k])
                # tail: hop 2 of the last D block
                for sp in range(spb):
                    for sup in emit_chains(2, (nrh - 1) * spb + sp):
                        emit_super_mms(*sup)

            # ---------------- Phase F: fuse + residual ----------------
            FR = min(512, r)
            nrf = r // FR
            with (
                tc.tile_pool(name="pf", bufs=1) as pf,
                tc.tile_pool(name="psF", bufs=2, space="PSUM") as psF,
                tc.tile_pool(name="evFp", bufs=2) as evict,
            ):
                hops_sb = pf.tile([P, nhc, r], f16, tag="hops_sb")
                for i in range(hop):
                    nc.gpsimd.dma_start(
                        hops_sb[:, i * nic:(i + 1) * nic, :],
                        hops_d[i][:, :].rearrange("(k p) q -> p k q", p=P),
                    )
                xres_sb = pf.tile([P, nxc, r], f16, tag="xres_sb")
                nc.gpsimd.dma_start(
                    xres_sb[:],
                    x16[0:ic, 0:r].rearrange("(k p) q -> p k q", p=P),
                )
                xp_sb = pf.tile([P, nic, r], f16, tag="xp_sb")

                for o in range(nic):
                    for rf in range(nrf):
                        ps = psF.tile([P, FR], f32, tag="psF")
                        for k in range(nhc):
                            nc.tensor.matmul(
                                ps[:],
                                fuseT_sb[:, k, o * P:(o + 1) * P],
                                hops_sb[:, k, rf * FR:(rf + 1) * FR],
                                start=(k == 0),
                                stop=(k == nhc - 1),
                            )
                        nc.scalar.activation(
                            xp_sb[:, o, rf * FR:(rf + 1) * FR],
                            ps[:],
                            AF.Identity,
                            bias=bias_sb[:, C_FUSE + o:C_FUSE + o + 1],
                        )

                for o in range(ncout):
                    for rf in range(nrf):
                        ps = psF.tile([P, FR], f32, tag="psF")
                        for k in range(nxc):
                            nc.tensor.matmul(
                                ps[:],
                                resT_sb[:, k, o * P:(o + 1) * P],
                                xres_sb[:, k, rf * FR:(rf + 1) * FR],
                                start=(k == 0),
                                stop=False,
                            )
                        for k in range(nic):
                            nc.tensor.matmul(
                                ps[:],
                                resT_sb[:, nxc + k, o * P:(o + 1) * P],
                                xp_sb[:, k, rf * FR:(rf + 1) * FR],
                                start=False,
                                stop=(k == nic - 1),
                            )
                        ev = evict.tile([P, FR], f32, tag="evF")
                        nc.scalar.activation(
                            ev[:],
                            ps[:],
                            AF.Identity,
                            bias=bias_sb[:, C_RES + o:C_RES + o + 1],
                        )
                        nc.sync.dma_start(
                            out[o * P:(o + 1) * P, rf * FR:(rf + 1) * FR], ev[:]
                        )

    nc.compile()
    return nc


def _host_prep(inputs, cin, ic, n, r, hop, eps):
    """Fold BN into weights; build per-core input maps."""

    def f32(a):
        return np.ascontiguousarray(np.asarray(a, dtype=np.float32))

    x = f32(inputs["x"])
    B = x.shape[0]
    xf = x.reshape(B, cin, n)

    s4 = float(ic) ** -0.25
    inv1 = 1.0 / np.sqrt(f32(inputs["bn1_v"]) + eps) * f32(inputs["bn1_g"])
    w1_eff = (inv1[:, None] * f32(inputs["w1_w"])) * s4
    b1_eff = (f32(inputs["w1_b"]) * inv1 + f32(inputs["bn1_b"])
              - f32(inputs["bn1_m"]) * inv1) * s4

    invf = 1.0 / np.sqrt(f32(inputs["bnf_v"]) + eps) * f32(inputs["bnf_g"])
    fuse_eff = invf[:, None] * f32(inputs["fuse_w"])
    fuse_b_eff = (f32(inputs["fuse_b"]) * invf + f32(inputs["bnf_b"])
                  - f32(inputs["bnf_m"]) * invf)

    invr = 1.0 / np.sqrt(f32(inputs["bnr_v"]) + eps) * f32(inputs["bnr_g"])
    res_eff = invr[:, None] * f32(inputs["res_w"])
    res_b_eff = (f32(inputs["res_b"]) * invr + f32(inputs["bnr_b"])
                 - f32(inputs["bnr_m"]) * invr)

    delta = float(np.asarray(inputs["delta"]).reshape(-1)[0])
    if delta <= 0.0:
        thr = -3.0e38
    elif delta >= 1.0:
        thr = 3.0e38
    else:
        thr = float(np.log(delta / (1.0 - delta)))

    nic = ic // P
    ncout = cin // P
    nbias = nic + hop * nic + nic + ncout
    bias_pack = np.zeros((P, nbias), np.float32)
    col = 0
    for oc in range(nic):
        bias_pack[:, col] = b1_eff[oc * P:(oc + 1) * P]
        col += 1
    hop_b = f32(inputs["hop_b"])
    for i in range(hop):
        for oc in range(nic):
            bias_pack[:, col] = hop_b[i, oc * P:(oc + 1) * P]
            col += 1
    for oc in range(nic):
        bias_pack[:, col] = fuse_b_eff[oc * P:(oc + 1) * P]
        col += 1
    for oc in range(ncout):
        bias_pack[:, col] = res_b_eff[oc * P:(oc + 1) * P]
        col += 1

    shared = {
        "w1T": np.ascontiguousarray(w1_eff.T).astype(np.float16),
        "nodeT": np.ascontiguousarray(f32(inputs["node_w"]).T).astype(
            np.float16
        ),
        "nbrow": f32(inputs["node_b"]).reshape(1, ic).astype(np.float16),
        "hopT": np.ascontiguousarray(
            f32(inputs["hop_w"]).transpose(0, 2, 1)
        ).astype(np.float16),
        "fuseT": np.ascontiguousarray(fuse_eff.T).astype(np.float16),
        "resT": np.ascontiguousarray(res_eff.T).astype(np.float16),
        "biases": bias_pack,
    }

    n_cores = (B * n) // r
    halves = n // r
    in_maps = []
    for c in range(n_cores):
        b, h = c // halves, c % halves
        perm = (np.arange(n) + h * r) % n
        m = dict(shared)
        m["x16"] = np.ascontiguousarray(xf[b][:, perm]).astype(np.float16)
        in_maps.append(m)
    return in_maps, thr


_BUILD_CACHE = {}


def kernel(**inputs):
    from concourse import bass_utils

    cin, ic, hop, eps = 512, 256, 3, 1e-5
    x = np.asarray(inputs["x"])
    B, _, H, W = x.shape
    n = H * W
    n_cores = 8
    r = (B * n) // n_cores
    halves = n // r

    in_maps, thr = _host_prep(inputs, cin, ic, n, r, hop, eps)

    key = (cin, ic, n, r, hop, thr)
    if key not in _BUILD_CACHE:
        _BUILD_CACHE[key] = _build(cin, ic, n, r, hop, thr)
    nc = _BUILD_CACHE[key]

    res = bass_utils.run_bass_kernel_spmd(nc, in_maps, core_ids=list(range(n_cores)))

    out = np.empty((B, cin, n), np.float32)
    for c in range(n_cores):
        b, h = c // halves, c % halves
        out[b][:, h * r:(h + 1) * r] = res.results[c]["out"]
    return out.reshape(B, cin, H, W).astype(x.dtype)


# revision 27
# speedup vs baseline: 1.0117x; 1.0117x over previous
"""Trainium2 Bass kernel for nn_HA_unit (gnn_message_passing).

Math (per batch b, N = H*W spatial positions):
  wfeat = BN1(w1 @ x)                       [IC, N]   (BN folded on host)
  iw    = wfeat^T wfeat * IC^-0.5           [N, N]    symmetric
  nodes = node_w @ x + node_b               [IC, N]   (kept as [N, IC])
  b0    = (sigmoid(iw) >= delta)            [N, N]    binary, symmetric
  bh_k  = b0^k  (k = 1, 2, 3)               exact integer counts
  hop_k = hopw_k @ (softmax(bh_k o iw) @ nodes)^T + hopb_k
  xp    = BNf(fuse_w @ concat(hops))
  out   = BNr(res_w @ concat(x[:IC], xp))

Sharding: 8 cores = 4 batches x 2 halves of N. Core (b, h) receives x[b]
with spatial positions rolled by h*N/2 so that its rows are always 0..N/2-1
(identical SPMD program, data-only difference). Each core computes the full
symmetric b0 locally (no collectives); the b0^2 / b0^3 matmuls are sharded
by output rows. Binary/int matmul operands are exact in bf16/f32.

Perf structure:
  - b0^2 (phase C) runs fp8 DoubleRow (2 K-planes per matmul).
  - bh2^T for phase D is produced with xbar DMA transposes, not PE.
  - DMA issue queues are split: streaming loads go through SWDGE
    (gpsimd), xbar transposes through ACT's HWDGE ring, stores through
    SP's — the SP ring alone saturates otherwise.
  - phase D's PSUM eviction is fused with the elementwise iw product, so
    it writes hop-2 softmax logits directly (no bh3 round trip).
  - softmax work is emitted interleaved with D's matmul blocks: the PE
    stream stays dense while DVE/ACT chew the softmax chains.
  - hop0 logits are bounded (|iw| <~ 6) so its max-reduce is skipped.
"""

import sys
from contextlib import ExitStack

sys.path.insert(0, "/opt/trn_rl_repo")

import numpy as np

P = 128


def _build(cin, ic, n, r, hop, thr):
    from concourse import bass, tile, bacc
    import concourse.mybir as mybir

    f32 = mybir.dt.float32
    f16 = mybir.dt.float16
    fp8 = mybir.dt.float8e4
    AF = mybir.ActivationFunctionType
    ALU = mybir.AluOpType
    AX = mybir.AxisListType
    DR = mybir.MatmulPerfMode.DoubleRow

    ncin = cin // P          # K-chunks over input channels
    nic = ic // P            # chunks over inter channels
    nkn = n // P             # K-chunks over N
    nrt = r // P             # our row tiles
    FB = min(512, n)         # free-dim blocking
    nfb = n // FB
    hc = hop * ic
    nhc = hc // P
    cout = cin
    ncout = cout // P
    nxc = ic // P            # x residual slice chunks (x[:ic])

    SB = 2                   # row tiles per softmax super-block
    nsb = nrt // SB          # super-blocks (8)
    RH = min(512, r)         # D row-block
    nrh = r // RH            # 4
    nrq = RH // P            # 4
    spb = nsb // nrh         # E super-blocks per D block (2)

    # bias_pack columns: [b1(nic) | hop(hop*nic) | fuse(nic) | res(ncout)]
    C_B1 = 0
    C_HOP = C_B1 + nic
    C_FUSE = C_HOP + hop * nic
    C_RES = C_FUSE + nic
    NBIAS = C_RES + ncout

    nc = bacc.Bacc("TRN2", target_bir_lowering=False, debug=False)

    x16 = nc.dram_tensor("x16", [cin, n], f16, kind="ExternalInput")
    w1T = nc.dram_tensor("w1T", [cin, ic], f16, kind="ExternalInput")
    nodeT = nc.dram_tensor("nodeT", [cin, ic], f16, kind="ExternalInput")
    nbrow = nc.dram_tensor("nbrow", [1, ic], f16, kind="ExternalInput")
    hopT = nc.dram_tensor("hopT", [hop, ic, ic], f16, kind="ExternalInput")
    fuseT = nc.dram_tensor("fuseT", [hc, ic], f16, kind="ExternalInput")
    resT = nc.dram_tensor("resT", [2 * ic, cout], f16, kind="ExternalInput")
    biases = nc.dram_tensor("biases", [P, NBIAS], f32, kind="ExternalInput")
    out = nc.dram_tensor("out", [cout, r], f32, kind="ExternalOutput")

    with tile.TileContext(nc) as tc:
        with (
            tc.tile_pool(name="dram", bufs=1, space="DRAM") as dpool,
            tc.tile_pool(name="consts", bufs=1) as consts,
            tc.tile_pool(name="pf_const", bufs=1) as pfc,
        ):
            BND = n // 4
            b0qb = [
                dpool.tile([BND, n], fp8, tag=f"b0q{j}", name=f"b0q{j}")
                for j in range(4)
            ]
            kpb = BND // P  # k-chunks per band (8)
            iwq = dpool.tile([r, n], f32, tag="iwq")
            bh2h = dpool.tile([r, n], f16, tag="bh2h")
            bh2T = dpool.tile([n, r], f16, tag="bh2T")
            lg3d = dpool.tile([r, n], f32, tag="lg3d")
            nodes_d = dpool.tile([n, ic], f16, tag="nodes_d")
            hops_d = [
                dpool.tile([ic, r], f16, tag=f"hops_d{i}", name=f"hops_d{i}")
                for i in range(hop)
            ]

            bias_sb = consts.tile([P, NBIAS], f32, tag="bias_sb")
            nc.sync.dma_start(bias_sb[:], biases[:])
            ones1 = consts.tile([1, P], f16, tag="ones1")
            nc.vector.memset(ones1[:], 1.0)
            nbrow_sb = consts.tile([1, ic], f16, tag="nbrow_sb")
            nc.sync.dma_start(nbrow_sb[:], nbrow[:])

            # ---------------- Phase A: wfeat + nodes ----------------
            with (
                tc.tile_pool(name="pa", bufs=1) as pa,
                tc.tile_pool(name="psA", bufs=2, space="PSUM") as psA,
                tc.tile_pool(name="evA", bufs=2) as evict,
                tc.tile_pool(name="rowA", bufs=2) as rowp,
            ):
                x_sb = pa.tile([P, ncin, n], f16, tag="x_sb")
                for k in range(ncin):
                    nc.gpsimd.dma_start(
                        x_sb[:, k, :],
                        x16[k * P:(k + 1) * P, :],
                    )
                w1T_sb = pa.tile([P, ncin, ic], f16, tag="w1T_sb")
                nc.gpsimd.dma_start(
                    w1T_sb[:], w1T[:, :].rearrange("(k p) o -> p k o", p=P)
                )
                nodeT_sb = pa.tile([P, ncin, ic], f16, tag="nodeT_sb")
                nc.gpsimd.dma_start(
                    nodeT_sb[:], nodeT[:, :].rearrange("(k p) o -> p k o", p=P)
                )
                wf_sb = pa.tile([P, nic, n], f16, tag="wf_sb")

                for oc in range(nic):
                    for f in range(nfb):
                        ps = psA.tile([P, FB], f32, tag=f"ps{f % 2}")
                        for k in range(ncin):
                            nc.tensor.matmul(
                                ps[:],
                                w1T_sb[:, k, oc * P:(oc + 1) * P],
                                x_sb[:, k, f * FB:(f + 1) * FB],
                                start=(k == 0),
                                stop=(k == ncin - 1),
                            )
                        nc.scalar.activation(
                            wf_sb[:, oc, f * FB:(f + 1) * FB],
                            ps[:],
                            AF.Identity,
                            bias=bias_sb[:, C_B1 + oc:C_B1 + oc + 1],
                        )

                for nt in range(nkn):
                    ps = psA.tile([P, ic], f32, tag="psn")
                    for k in range(ncin):
                        nc.tensor.matmul(
                            ps[:],
                            x_sb[:, k, nt * P:(nt + 1) * P],
                            nodeT_sb[:, k, :],
                            start=(k == 0),
                            stop=False,
                        )
                    nc.tensor.matmul(
                        ps[:], ones1[:], nbrow_sb[:], start=False, stop=True
                    )
                    ev = evict.tile([P, ic], f16, tag="evn")
                    nc.vector.tensor_copy(ev[:], ps[:])
                    nc.sync.dma_start(nodes_d[nt * P:(nt + 1) * P, :], ev[:])

                # ---------------- Phase B: iw + b0 ----------------
                # evictions accumulate into full row blocks, one store per
                # row block, to keep the SP DMA ring shallow.
                for pc in range(nkn):
                    b0row = rowp.tile([P, n], fp8, tag="b0row")
                    iwrow = None
                    if pc * P < r:
                        iwrow = rowp.tile([P, n], f32, tag="iwrow")
                    for f in range(nfb):
                        ps = psA.tile([P, FB], f32, tag=f"ps{f % 2}")
                        for k in range(nic):
                            nc.tensor.matmul(
                                ps[:],
                                wf_sb[:, k, pc * P:(pc + 1) * P],
                                wf_sb[:, k, f * FB:(f + 1) * FB],
                                start=(k == 0),
                                stop=(k == nic - 1),
                            )
                        nc.vector.tensor_scalar(
                            b0row[:, f * FB:(f + 1) * FB], ps[:], thr, None,
                            op0=ALU.is_ge,
                        )
                        if iwrow is not None:
                            nc.scalar.activation(
                                iwrow[:, f * FB:(f + 1) * FB], ps[:], AF.Copy
                            )
                    nc.sync.dma_start(
                        b0qb[pc // kpb][(pc % kpb) * P:(pc % kpb + 1) * P, :],
                        b0row[:],
                    )
                    if iwrow is not None:
                        nc.sync.dma_start(
                            iwq[pc * P:(pc + 1) * P, :], iwrow[:]
                        )

            # ------- Phase C: bh2 = b0 @ b0 (fp8 DoubleRow, exact f32) -------
            with (
                tc.tile_pool(name="pc", bufs=1) as pcp,
                tc.tile_pool(name="pc_rhs", bufs=2) as pcr,
                tc.tile_pool(name="psC", bufs=2, space="PSUM") as psC,
                tc.tile_pool(name="evCp", bufs=2) as evictC,
            ):
                lh = pcp.tile([P, nkn, r], fp8, tag="lh")
                for j in range(4):
                    nc.gpsimd.dma_start(
                        lh[:, j * kpb:(j + 1) * kpb, :],
                        b0qb[j][:, 0:r].rearrange("(k p) q -> p k q", p=P),
                    )
                for mc in range(nfb):
                    rt_ = pcr.tile([P, nkn, FB], fp8, tag="rhsC")
                    for j in range(4):
                        nc.gpsimd.dma_start(
                            rt_[:, j * kpb:(j + 1) * kpb, :],
                            b0qb[j][:, mc * FB:(mc + 1) * FB].rearrange(
                                "(k p) q -> p k q", p=P
                            ),
                        )
                    for rq in range(nrt):
                        ps = psC.tile(
                            [P, FB], f32, tag=f"psC{rq % 4}", bufs=1
                        )
                        for k in range(0, nkn, 2):
                            nc.tensor.matmul(
                                ps[:],
                                lh[:, k:k + 2, rq * P:(rq + 1) * P],
                                rt_[:, k:k + 2, :],
                                start=(k == 0),
                                stop=(k == nkn - 2),
                                perf_mode=DR,
                            )
                        ev = evictC.tile(
                            [P, FB], f16, tag=f"evC{rq % 4}", bufs=1
                        )
                        nc.vector.tensor_copy(ev[:], ps[:])
                        rg = rq * P
                        nc.sync.dma_start(
                            bh2h[rg:rg + P, mc * FB:(mc + 1) * FB], ev[:]
                        )
                        if rq % 4 == 0:
                            tTb = evictC.tile(
                                [P, FB // P, 4 * P], f16,
                                tag=f"tT{(rq // 4) % 2}", bufs=1,
                            )
                        nc.scalar.dma_start(
                            tTb[:, :, (rq % 4) * P:(rq % 4 + 1) * P],
                            ev[:],
                            transpose=True,
                        )
                        if rq % 4 == 3:
                            # one contiguous 1KB-per-row store per 4 row
                            # tiles -- per-tile stores are 256B-descriptor
                            # scatters that clog the SP DMA ring
                            nc.sync.dma_start(
                                bh2T[
                                    mc * FB:(mc + 1) * FB, rg - 3 * P:rg + P
                                ].rearrange("(j p) q -> p j q", p=P),
                                tTb[:],
                            )

            # ---- Phases D (bh3 logits) + E (softmax hops), interleaved ----
            with ExitStack() as stk:
                ec = stk.enter_context
                pec = ec(tc.tile_pool(name="pe_const", bufs=1))
                peiw = ec(tc.tile_pool(name="pe_iw", bufs=1))
                pebh = ec(tc.tile_pool(name="pe_bh", bufs=1))
                pelg = ec(tc.tile_pool(name="pe_lg", bufs=1))
                pept = ec(tc.tile_pool(name="pe_pt", bufs=2))
                pesT = ec(tc.tile_pool(name="pe_sT", bufs=2))
                peo = ec(tc.tile_pool(name="pe_out", bufs=2))
                psE = ec(tc.tile_pool(name="psE", bufs=2, space="PSUM"))
                psH = ec(tc.tile_pool(name="psH", bufs=2, space="PSUM"))
                pdl = ec(tc.tile_pool(name="pd_lhs", bufs=1))
                pdr = ec(tc.tile_pool(name="pd_rhs", bufs=2))
                pdiw = ec(tc.tile_pool(name="pd_iw", bufs=2))
                psD = ec(tc.tile_pool(name="psD", bufs=2, space="PSUM"))
                evictD = ec(tc.tile_pool(name="evDp", bufs=2))
                fuseT_sb = pfc.tile([P, nhc, ic], f16, tag="fuseT_sb")
                nc.gpsimd.dma_start(
                    fuseT_sb[:], fuseT[:, :].rearrange("(k p) o -> p k o", p=P)
                )
                resT_sb = pfc.tile([P, 2 * nic, cout], f16, tag="resT_sb")
                nc.gpsimd.dma_start(
                    resT_sb[:], resT[:, :].rearrange("(k p) o -> p k o", p=P)
                )
                nodes_sb = pec.tile([P, nkn, ic], f16, tag="nodes_sb")
                nc.gpsimd.dma_start(
                    nodes_sb[:], nodes_d[:, :].rearrange("(t p) c -> p t c", p=P)
                )
                hopT_sb = pec.tile([P, hop, nic, ic], f16, tag="hopT_sb")
                for i in range(hop):
                    nc.gpsimd.dma_start(
                        hopT_sb[:, i, :, :],
                        hopT[i, :, :].rearrange("(k p) o -> p k o", p=P),
                    )

                def emit_chain(i, rt, sT, sl, iwt=None):
                    """Softmax chain for hop i, row tile rt; writes the
                    transposed, normalized probabilities into
                    sT[:, :, sl*P:(sl+1)*P]. Hops 0/1 share one iw row
                    load; hop0 uses ACT relu when thr == 0 (b0 o iw ==
                    relu(iw)) so no b0 row load or DVE mul is needed."""
                    if i == 2:
                        lg = pelg.tile([P, n], f32, tag="lg")
                        nc.gpsimd.dma_start(
                            lg[:], lg3d[rt * P:(rt + 1) * P, :]
                        )
                    elif i == 0 and thr == 0.0:
                        lg = pelg.tile([P, n], f32, tag="lg")
                        nc.scalar.activation(lg[:], iwt[:], AF.Relu)
                    else:
                        if i == 0:
                            bht = pebh.tile([P, n], fp8, tag="bh")
                            nc.gpsimd.dma_start(
                                bht[:],
                                b0qb[rt // kpb][
                                    (rt % kpb) * P:(rt % kpb + 1) * P, :
                                ],
                            )
                        else:
                            bht = pebh.tile([P, n], f16, tag="bh")
                            nc.gpsimd.dma_start(
                                bht[:], bh2h[rt * P:(rt + 1) * P, :]
                            )
                        lg = pelg.tile([P, n], f32, tag="lg")
                        nc.vector.tensor_mul(lg[:], iwt[:], bht[:])
                    nmax = None
                    if i > 0:
                        nmax = peo.tile([P, 1], f32, tag="nmax")
                        nc.vector.tensor_reduce(
                            nmax[:], lg[:], axis=AX.X, op=ALU.max, negate=True
                        )
                    zt = peo.tile([P, 1], f32, tag="zt")
                    pt = pept.tile([P, n], f16, tag="pt")
                    if nmax is not None:
                        nc.scalar.activation(
                            pt[:], lg[:], AF.Exp, bias=nmax[:], accum_out=zt[:]
                        )
                    else:
                        nc.scalar.activation(
                            pt[:], lg[:], AF.Exp, accum_out=zt[:]
                        )
                    rz = peo.tile([P, 1], f32, tag="rz")
                    nc.vector.reciprocal(rz[:], zt[:])
                    st = pept.tile([P, n], f16, tag="pt")
                    nc.scalar.activation(st[:], pt[:], AF.Copy, scale=rz[:])
                    nc.scalar.dma_start(
                        sT[:, :, sl * P:(sl + 1) * P], st[:], transpose=True
                    )

                def emit_chains(i, s):
                    """Emit the softmax chains for a todo entry. i == 01
                    emits hops 0 and 1 together, sharing one iw row load
                    per row tile; returns the super descriptors."""
                    if i == 1:
                        sT0 = pesT.tile([P, nkn, SB * P], f16, tag="sT")
                        sT1 = pesT.tile([P, nkn, SB * P], f16, tag="sT")
                        for sl in range(SB):
                            rt = s * SB + sl
                            iwt = peiw.tile([P, n], f32, tag="iwt")
                            nc.gpsimd.dma_start(
                                iwt[:], iwq[rt * P:(rt + 1) * P, :]
                            )
                            emit_chain(0, rt, sT0, sl, iwt)
                            emit_chain(1, rt, sT1, sl, iwt)
                        return [(0, s, sT0), (1, s, sT1)]
                    sT = pesT.tile([P, nkn, SB * P], f16, tag="sT")
                    for sl in range(SB):
                        emit_chain(i, s * SB + sl, sT, sl)
                    return [(i, s, sT)]

                def emit_super_mms(i, s, sT):
                    """t = s @ nodes and the hop conv for super-block s of
                    hop i (SB row tiles at once, FD=SB*P). Evictions stay
                    off ACT so the in-order ACT queue only carries the
                    softmax chains."""
                    W = SB * P
                    tcb = peo.tile([P, nic, W], f16, tag=f"tcb{s % 2}")
                    for c in range(nic):
                        ps = psE.tile([P, W], f32, tag="psE")
                        for j in range(nkn):
                            nc.tensor.matmul(
                                ps[:],
                                nodes_sb[:, j, c * P:(c + 1) * P],
                                sT[:, j, :],
                                start=(j == 0),
                                stop=(j == nkn - 1),
                            )
                        nc.vector.tensor_copy(tcb[:, c, :], ps[:])
                    hv = peo.tile([P, nic, W], f16, tag=f"hv{s % 2}")
                    for o in range(nic):
                        ph = psH.tile([P, W], f32, tag="psH")
                        for c in range(nic):
                            nc.tensor.matmul(
                                ph[:],
                                hopT_sb[:, i, c, o * P:(o + 1) * P],
                                tcb[:, c, :],
                                start=(c == 0),
                                stop=(c == nic - 1),
                            )
                        nc.vector.tensor_scalar(
                            hv[:, o, :],
                            ph[:],
                            bias_sb[
                                :, C_HOP + i * nic + o:C_HOP + i * nic + o + 1
                            ],
                            None,
                            op0=ALU.add,
                        )
                    nc.sync.dma_start(
                        hops_d[i][:, s * W:(s + 1) * W].rearrange(
                            "(o p) q -> p o q", p=P
                        ),
                        hv[:],
                    )

                # Interleave schedule per D block b (nfb=8 slots):
                #   slot m: emit chains for pending hop-super m (if any),
                #           then D's mc=m matmul group,
                #           then consume super m-offset's matmuls.
                # Chains get >=2 D-groups (~50us) of DVE/ACT headroom
                # before their PE matmuls, and every engine queue stays
                # aligned with consumption order.
                for b in range(nrh):
                    lh = pdl.tile([P, nkn, RH], f16, tag="lhD")
                    nc.gpsimd.dma_start(
                        lh[:],
                        bh2T[:, b * RH:(b + 1) * RH].rearrange(
                            "(k p) q -> p k q", p=P
                        ),
                    )
                    todo = []
                    for sp in range(spb):
                        todo.append((1, b * spb + sp))
                    if b > 0:
                        for sp in range(spb):
                            todo.append((2, (b - 1) * spb + sp))
                    nsup = 2 * spb + (spb if b > 0 else 0)
                    offset = nfb - nsup
                    filled = []
                    ncons = 0
                    for mc in range(nfb):
                        if mc >= offset and ncons < len(filled):
                            emit_super_mms(*filled[ncons])
                            ncons += 1
                        rt_ = pdr.tile([P, nkn, FB], fp8, tag="rhsD")
                        for j in range(4):
                            nc.gpsimd.dma_start(
                                rt_[:, j * kpb:(j + 1) * kpb, :],
                                b0qb[j][:, mc * FB:(mc + 1) * FB].rearrange(
                                    "(k p) q -> p k q", p=P
                                ),
                            )
                        for rq in range(nrq):
                            ps = psD.tile(
                                [P, FB], f32, tag=f"psD{rq % 4}", bufs=1
                            )
                            for k in range(nkn):
                                nc.tensor.matmul(
                                    ps[:],
                                    lh[:, k, rq * P:(rq + 1) * P],
                                    rt_[:, k, :],
                                    start=(k == 0),
                                    stop=(k == nkn - 1),
                                )
                            rg = b * RH + rq * P
                            iwd = pdiw.tile(
                                [P, FB], f32, tag=f"iwd{rq % 4}", bufs=1
                            )
                            nc.gpsimd.dma_start(
                                iwd[:],
                                iwq[rg:rg + P, mc * FB:(mc + 1) * FB],
                            )
                            ev = evictD.tile(
                                [P, FB], f32, tag=f"evD{rq % 4}", bufs=1
                            )
                            nc.vector.tensor_mul(ev[:], ps[:], iwd[:])
                            nc.sync.dma_start(
                                lg3d[rg:rg + P, mc * FB:(mc + 1) * FB], ev[:]
                            )
                        if mc < len(todo):
                            filled.extend(emit_chains(*todo[mc]))
                    for k in range(ncons, len(filled)):
                        emit_super_mms(*filled[k])
                # tail: hop 2 of the last D block
                for sp in range(spb):
                    for sup in emit_chains(2, (nrh - 1) * spb + sp):
                        emit_super_mms(*sup)

            # ---------------- Phase F: fuse + residual ----------------
            FR = min(512, r)
            nrf = r // FR
            with (
                tc.tile_pool(name="pf", bufs=1) as pf,
                tc.tile_pool(name="psF", bufs=2, space="PSUM") as psF,
                tc.tile_pool(name="evFp", bufs=2) as evict,
            ):
                hops_sb = pf.tile([P, nhc, r], f16, tag="hops_sb")
                for i in range(hop):
                    nc.gpsimd.dma_start(
                        hops_sb[:, i * nic:(i + 1) * nic, :],
                        hops_d[i][:, :].rearrange("(k p) q -> p k q", p=P),
                    )
                xres_sb = pf.tile([P, nxc, r], f16, tag="xres_sb")
                nc.gpsimd.dma_start(
                    xres_sb[:],
                    x16[0:ic, 0:r].rearrange("(k p) q -> p k q", p=P),
                )
                xp_sb = pf.tile([P, nic, r], f16, tag="xp_sb")

                for o in range(nic):
                    for rf in range(nrf):
                        ps = psF.tile([P, FR], f32, tag="psF")
                        for k in range(nhc):
                            nc.tensor.matmul(
                                ps[:],
                                fuseT_sb[:, k, o * P:(o + 1) * P],
                                hops_sb[:, k, rf * FR:(rf + 1) * FR],
                                start=(k == 0),
                                stop=(k == nhc - 1),
                            )
                        nc.scalar.activation(
                            xp_sb[:, o, rf * FR:(rf + 1) * FR],
                            ps[:],
                            AF.Identity,
                            bias=bias_sb[:, C_FUSE + o:C_FUSE + o + 1],
                        )

                for o in range(ncout):
                    for rf in range(nrf):
                        ps = psF.tile([P, FR], f32, tag="psF")
                        for k in range(nxc):
                            nc.tensor.matmul(
                                ps[:],
                                resT_sb[:, k, o * P:(o + 1) * P],
                                xres_sb[:, k, rf * FR:(rf + 1) * FR],
                                start=(k == 0),
                                stop=False,
                            )
                        for k in range(nic):
                            nc.tensor.matmul(
                                ps[:],
                                resT_sb[:, nxc + k, o * P:(o + 1) * P],
                                xp_sb[:, k, rf * FR:(rf + 1) * FR],
                                start=False,
                                stop=(k == nic - 1),
                            )
                        ev = evict.tile([P, FR], f32, tag="evF")
                        nc.scalar.activation(
                            ev[:],
                            ps[:],
                            AF.Identity,
                            bias=bias_sb[:, C_RES + o:C_RES + o + 1],
                        )
                        nc.sync.dma_start(
                            out[o * P:(o + 1) * P, rf * FR:(rf + 1) * FR], ev[:]
                        )

    nc.compile()
    return nc


def _host_prep(inputs, cin, ic, n, r, hop, eps):
    """Fold BN into weights; build per-core input maps."""

    def f32(a):
        return np.ascontiguousarray(np.asarray(a, dtype=np.float32))

    x = f32(inputs["x"])
    B = x.shape[0]
    xf = x.reshape(B, cin, n)

    s4 = float(ic) ** -0.25
    inv1 = 1.0 / np.sqrt(f32(inputs["bn1_v"]) + eps) * f32(inputs["bn1_g"])
    w1_eff = (inv1[:, None] * f32(inputs["w1_w"])) * s4
    b1_eff = (f32(inputs["w1_b"]) * inv1 + f32(inputs["bn1_b"])
              - f32(inputs["bn1_m"]) * inv1) * s4

    invf = 1.0 / np.sqrt(f32(inputs["bnf_v"]) + eps) * f32(inputs["bnf_g"])
    fuse_eff = invf[:, None] * f32(inputs["fuse_w"])
    fuse_b_eff = (f32(inputs["fuse_b"]) * invf + f32(inputs["bnf_b"])
                  - f32(inputs["bnf_m"]) * invf)

    invr = 1.0 / np.sqrt(f32(inputs["bnr_v"]) + eps) * f32(inputs["bnr_g"])
    res_eff = invr[:, None] * f32(inputs["res_w"])
    res_b_eff = (f32(inputs["res_b"]) * invr + f32(inputs["bnr_b"])
                 - f32(inputs["bnr_m"]) * invr)

    delta = float(np.asarray(inputs["delta"]).reshape(-1)[0])
    if delta <= 0.0:
        thr = -3.0e38
    elif delta >= 1.0:
        thr = 3.0e38
    else:
        thr = float(np.log(delta / (1.0 - delta)))

    nic = ic // P
    ncout = cin // P
    nbias = nic + hop * nic + nic + ncout
    bias_pack = np.zeros((P, nbias), np.float32)
    col = 0
    for oc in range(nic):
        bias_pack[:, col] = b1_eff[oc * P:(oc + 1) * P]
        col += 1
    hop_b = f32(inputs["hop_b"])
    for i in range(hop):
        for oc in range(nic):
            bias_pack[:, col] = hop_b[i, oc * P:(oc + 1) * P]
            col += 1
    for oc in range(nic):
        bias_pack[:, col] = fuse_b_eff[oc * P:(oc + 1) * P]
        col += 1
    for oc in range(ncout):
        bias_pack[:, col] = res_b_eff[oc * P:(oc + 1) * P]
        col += 1

    shared = {
        "w1T": np.ascontiguousarray(w1_eff.T).astype(np.float16),
        "nodeT": np.ascontiguousarray(f32(inputs["node_w"]).T).astype(
            np.float16
        ),
        "nbrow": f32(inputs["node_b"]).reshape(1, ic).astype(np.float16),
        "hopT": np.ascontiguousarray(
            f32(inputs["hop_w"]).transpose(0, 2, 1)
        ).astype(np.float16),
        "fuseT": np.ascontiguousarray(fuse_eff.T).astype(np.float16),
        "resT": np.ascontiguousarray(res_eff.T).astype(np.float16),
        "biases": bias_pack,
    }

    n_cores = (B * n) // r
    halves = n // r
    in_maps = []
    for c in range(n_cores):
        b, h = c // halves, c % halves
        perm = (np.arange(n) + h * r) % n
        m = dict(shared)
        m["x16"] = np.ascontiguousarray(xf[b][:, perm]).astype(np.float16)
        in_maps.append(m)
    return in_maps, thr


_BUILD_CACHE = {}


def kernel(**inputs):
    from concourse import bass_utils

    cin, ic, hop, eps = 512, 256, 3, 1e-5
    x = np.asarray(inputs["x"])
    B, _, H, W = x.shape
    n = H * W
    n_cores = 8
    r = (B * n) // n_cores
    halves = n // r

    in_maps, thr = _host_prep(inputs, cin, ic, n, r, hop, eps)

    key = (cin, ic, n, r, hop, thr)
    if key not in _BUILD_CACHE:
        _BUILD_CACHE[key] = _build(cin, ic, n, r, hop, thr)
    nc = _BUILD_CACHE[key]

    res = bass_utils.run_bass_kernel_spmd(nc, in_maps, core_ids=list(range(n_cores)))

    out = np.empty((B, cin, n), np.float32)
    for c in range(n_cores):
        b, h = c // halves, c % halves
        out[b][:, h * r:(h + 1) * r] = res.results[c]["out"]
    return out.reshape(B, cin, H, W).astype(x.dtype)


# revision 28
# speedup vs baseline: 1.0355x; 1.0235x over previous
"""Trainium2 Bass kernel for nn_HA_unit (gnn_message_passing).

Math (per batch b, N = H*W spatial positions):
  wfeat = BN1(w1 @ x)                       [IC, N]   (BN folded on host)
  iw    = wfeat^T wfeat * IC^-0.5           [N, N]    symmetric
  nodes = node_w @ x + node_b               [IC, N]   (kept as [N, IC])
  b0    = (sigmoid(iw) >= delta)            [N, N]    binary, symmetric
  bh_k  = b0^k  (k = 1, 2, 3)               exact integer counts
  hop_k = hopw_k @ (softmax(bh_k o iw) @ nodes)^T + hopb_k
  xp    = BNf(fuse_w @ concat(hops))
  out   = BNr(res_w @ concat(x[:IC], xp))

Sharding: 8 cores = 4 batches x 2 halves of N. Core (b, h) receives x[b]
with spatial positions rolled by h*N/2 so that its rows are always 0..N/2-1
(identical SPMD program, data-only difference). Each core computes the full
symmetric b0 locally (no collectives); the b0^2 / b0^3 matmuls are sharded
by output rows. Binary/int matmul operands are exact in bf16/f32.

Perf structure:
  - b0^2 (phase C) runs fp8 DoubleRow (2 K-planes per matmul).
  - bh2^T for phase D is produced with xbar DMA transposes, not PE.
  - DMA issue queues are split: streaming loads go through SWDGE
    (gpsimd), xbar transposes through ACT's HWDGE ring, stores through
    SP's — the SP ring alone saturates otherwise.
  - phase D's PSUM eviction is fused with the elementwise iw product, so
    it writes hop-2 softmax logits directly (no bh3 round trip).
  - softmax work is emitted interleaved with D's matmul blocks: the PE
    stream stays dense while DVE/ACT chew the softmax chains.
  - hop0 logits are bounded (|iw| <~ 6) so its max-reduce is skipped.
"""

import sys
from contextlib import ExitStack

sys.path.insert(0, "/opt/trn_rl_repo")

import numpy as np

P = 128


def _build(cin, ic, n, r, hop, thr):
    from concourse import bass, tile, bacc
    import concourse.mybir as mybir

    f32 = mybir.dt.float32
    f16 = mybir.dt.float16
    fp8 = mybir.dt.float8e4
    AF = mybir.ActivationFunctionType
    ALU = mybir.AluOpType
    AX = mybir.AxisListType
    DR = mybir.MatmulPerfMode.DoubleRow

    ncin = cin // P          # K-chunks over input channels
    nic = ic // P            # chunks over inter channels
    nkn = n // P             # K-chunks over N
    nrt = r // P             # our row tiles
    FB = min(512, n)         # free-dim blocking
    nfb = n // FB
    hc = hop * ic
    nhc = hc // P
    cout = cin
    ncout = cout // P
    nxc = ic // P            # x residual slice chunks (x[:ic])

    SB = 2                   # row tiles per softmax super-block
    nsb = nrt // SB          # super-blocks (8)
    RH = min(512, r)         # D row-block
    nrh = r // RH            # 4
    nrq = RH // P            # 4
    spb = nsb // nrh         # E super-blocks per D block (2)

    # bias_pack columns: [b1(nic) | hop(hop*nic) | fuse(nic) | res(ncout)]
    C_B1 = 0
    C_HOP = C_B1 + nic
    C_FUSE = C_HOP + hop * nic
    C_RES = C_FUSE + nic
    NBIAS = C_RES + ncout

    nc = bacc.Bacc("TRN2", target_bir_lowering=False, debug=False)

    x16 = nc.dram_tensor("x16", [cin, n], f16, kind="ExternalInput")
    w1T = nc.dram_tensor("w1T", [cin, ic], f16, kind="ExternalInput")
    nodeT = nc.dram_tensor("nodeT", [cin, ic], f16, kind="ExternalInput")
    nbrow = nc.dram_tensor("nbrow", [1, ic], f16, kind="ExternalInput")
    hopT = nc.dram_tensor("hopT", [hop, ic, ic], f16, kind="ExternalInput")
    fuseT = nc.dram_tensor("fuseT", [hc, ic], f16, kind="ExternalInput")
    resT = nc.dram_tensor("resT", [2 * ic, cout], f16, kind="ExternalInput")
    biases = nc.dram_tensor("biases", [P, NBIAS], f32, kind="ExternalInput")
    out = nc.dram_tensor("out", [cout, r], f32, kind="ExternalOutput")

    with tile.TileContext(nc) as tc:
        with (
            tc.tile_pool(name="dram", bufs=1, space="DRAM") as dpool,
            tc.tile_pool(name="consts", bufs=1) as consts,
            tc.tile_pool(name="pf_const", bufs=1) as pfc,
        ):
            BND = n // 4
            b0qb = [
                dpool.tile([BND, n], fp8, tag=f"b0q{j}", name=f"b0q{j}")
                for j in range(4)
            ]
            kpb = BND // P  # k-chunks per band (8)
            iwq = dpool.tile([r, n], f32, tag="iwq")
            bh2h = dpool.tile([r, n], f16, tag="bh2h")
            bh2T = dpool.tile([n, r], f16, tag="bh2T")
            lg3d = dpool.tile([r, n], f32, tag="lg3d")
            nodes_d = dpool.tile([n, ic], f16, tag="nodes_d")
            hops_d = [
                dpool.tile([ic, r], f16, tag=f"hops_d{i}", name=f"hops_d{i}")
                for i in range(hop)
            ]

            bias_sb = consts.tile([P, NBIAS], f32, tag="bias_sb")
            nc.sync.dma_start(bias_sb[:], biases[:])
            ones1 = consts.tile([1, P], f16, tag="ones1")
            nc.vector.memset(ones1[:], 1.0)
            nbrow_sb = consts.tile([1, ic], f16, tag="nbrow_sb")
            nc.sync.dma_start(nbrow_sb[:], nbrow[:])

            # ---------------- Phase A: wfeat + nodes ----------------
            with (
                tc.tile_pool(name="pa", bufs=1) as pa,
                tc.tile_pool(name="psA", bufs=2, space="PSUM") as psA,
                tc.tile_pool(name="evA", bufs=2) as evict,
                tc.tile_pool(name="rowA", bufs=2) as rowp,
            ):
                x_sb = pa.tile([P, ncin, n], f16, tag="x_sb")
                for k in range(ncin):
                    nc.gpsimd.dma_start(
                        x_sb[:, k, :],
                        x16[k * P:(k + 1) * P, :],
                    )
                w1T_sb = pa.tile([P, ncin, ic], f16, tag="w1T_sb")
                nc.gpsimd.dma_start(
                    w1T_sb[:], w1T[:, :].rearrange("(k p) o -> p k o", p=P)
                )
                nodeT_sb = pa.tile([P, ncin, ic], f16, tag="nodeT_sb")
                nc.gpsimd.dma_start(
                    nodeT_sb[:], nodeT[:, :].rearrange("(k p) o -> p k o", p=P)
                )
                wf_sb = pa.tile([P, nic, n], f16, tag="wf_sb")

                for oc in range(nic):
                    for f in range(nfb):
                        ps = psA.tile([P, FB], f32, tag=f"ps{f % 2}")
                        for k in range(ncin):
                            nc.tensor.matmul(
                                ps[:],
                                w1T_sb[:, k, oc * P:(oc + 1) * P],
                                x_sb[:, k, f * FB:(f + 1) * FB],
                                start=(k == 0),
                                stop=(k == ncin - 1),
                            )
                        nc.scalar.activation(
                            wf_sb[:, oc, f * FB:(f + 1) * FB],
                            ps[:],
                            AF.Identity,
                            bias=bias_sb[:, C_B1 + oc:C_B1 + oc + 1],
                        )

                for nt in range(nkn):
                    ps = psA.tile([P, ic], f32, tag="psn")
                    for k in range(ncin):
                        nc.tensor.matmul(
                            ps[:],
                            x_sb[:, k, nt * P:(nt + 1) * P],
                            nodeT_sb[:, k, :],
                            start=(k == 0),
                            stop=False,
                        )
                    nc.tensor.matmul(
                        ps[:], ones1[:], nbrow_sb[:], start=False, stop=True
                    )
                    ev = evict.tile([P, ic], f16, tag="evn")
                    nc.vector.tensor_copy(ev[:], ps[:])
                    nc.sync.dma_start(nodes_d[nt * P:(nt + 1) * P, :], ev[:])

                # ---------------- Phase B: iw + b0 ----------------
                # evictions accumulate into full row blocks, one store per
                # row block, to keep the SP DMA ring shallow.
                for pc in range(nkn):
                    b0row = rowp.tile([P, n], fp8, tag="b0row")
                    iwrow = None
                    if pc * P < r:
                        iwrow = rowp.tile([P, n], f32, tag="iwrow")
                    for f in range(nfb):
                        ps = psA.tile([P, FB], f32, tag=f"ps{f % 2}")
                        for k in range(nic):
                            nc.tensor.matmul(
                                ps[:],
                                wf_sb[:, k, pc * P:(pc + 1) * P],
                                wf_sb[:, k, f * FB:(f + 1) * FB],
                                start=(k == 0),
                                stop=(k == nic - 1),
                            )
                        nc.vector.tensor_scalar(
                            b0row[:, f * FB:(f + 1) * FB], ps[:], thr, None,
                            op0=ALU.is_ge,
                        )
                        if iwrow is not None:
                            nc.scalar.activation(
                                iwrow[:, f * FB:(f + 1) * FB], ps[:], AF.Copy
                            )
                    nc.sync.dma_start(
                        b0qb[pc // kpb][(pc % kpb) * P:(pc % kpb + 1) * P, :],
                        b0row[:],
                    )
                    if iwrow is not None:
                        nc.sync.dma_start(
                            iwq[pc * P:(pc + 1) * P, :], iwrow[:]
                        )

            # ------- Phase C: bh2 = b0 @ b0 (fp8 DoubleRow, exact f32) -------
            with (
                tc.tile_pool(name="pc", bufs=1) as pcp,
                tc.tile_pool(name="pc_rhs", bufs=2) as pcr,
                tc.tile_pool(name="psC", bufs=2, space="PSUM") as psC,
                tc.tile_pool(name="evCp", bufs=2) as evictC,
            ):
                lh = pcp.tile([P, nkn, r], fp8, tag="lh")
                for j in range(4):
                    nc.gpsimd.dma_start(
                        lh[:, j * kpb:(j + 1) * kpb, :],
                        b0qb[j][:, 0:r].rearrange("(k p) q -> p k q", p=P),
                    )
                for mc in range(nfb):
                    rt_ = pcr.tile([P, nkn, FB], fp8, tag="rhsC")
                    for j in range(4):
                        nc.gpsimd.dma_start(
                            rt_[:, j * kpb:(j + 1) * kpb, :],
                            b0qb[j][:, mc * FB:(mc + 1) * FB].rearrange(
                                "(k p) q -> p k q", p=P
                            ),
                        )
                    for rq2 in range(0, nrt, 2):
                        # Two accumulation groups interleaved at the
                        # instruction level: the PE alternates PSUM banks
                        # each matmul, so one group's eviction round-trip
                        # hides under the other group's matmul stream.
                        psa = psC.tile(
                            [P, FB], f32, tag=f"psC{rq2 % 4}", bufs=1
                        )
                        psb = psC.tile(
                            [P, FB], f32, tag=f"psC{(rq2 + 1) % 4}", bufs=1
                        )
                        for k in range(0, nkn, 2):
                            nc.tensor.matmul(
                                psa[:],
                                lh[:, k:k + 2, rq2 * P:(rq2 + 1) * P],
                                rt_[:, k:k + 2, :],
                                start=(k == 0),
                                stop=(k == nkn - 2),
                                perf_mode=DR,
                            )
                            nc.tensor.matmul(
                                psb[:],
                                lh[:, k:k + 2, (rq2 + 1) * P:(rq2 + 2) * P],
                                rt_[:, k:k + 2, :],
                                start=(k == 0),
                                stop=(k == nkn - 2),
                                perf_mode=DR,
                            )
                        for rq, ps in ((rq2, psa), (rq2 + 1, psb)):
                            ev = evictC.tile(
                                [P, FB], f16, tag=f"evC{rq % 4}", bufs=1
                            )
                            nc.vector.tensor_copy(ev[:], ps[:])
                            rg = rq * P
                            nc.sync.dma_start(
                                bh2h[rg:rg + P, mc * FB:(mc + 1) * FB], ev[:]
                            )
                            if rq % 4 == 0:
                                tTb = evictC.tile(
                                    [P, FB // P, 4 * P], f16,
                                    tag=f"tT{(rq // 4) % 2}", bufs=1,
                                )
                            nc.scalar.dma_start(
                                tTb[:, :, (rq % 4) * P:(rq % 4 + 1) * P],
                                ev[:],
                                transpose=True,
                            )
                            if rq % 4 == 3:
                                # one contiguous 1KB-per-row store per 4
                                # row tiles -- per-tile stores are
                                # 256B-descriptor scatters that clog the
                                # SP DMA ring
                                nc.sync.dma_start(
                                    bh2T[
                                        mc * FB:(mc + 1) * FB,
                                        rg - 3 * P:rg + P,
                                    ].rearrange("(j p) q -> p j q", p=P),
                                    tTb[:],
                                )

            # ---- Phases D (bh3 logits) + E (softmax hops), interleaved ----
            with ExitStack() as stk:
                ec = stk.enter_context
                pec = ec(tc.tile_pool(name="pe_const", bufs=1))
                peiw = ec(tc.tile_pool(name="pe_iw", bufs=1))
                pebh = ec(tc.tile_pool(name="pe_bh", bufs=1))
                pelg = ec(tc.tile_pool(name="pe_lg", bufs=1))
                pept = ec(tc.tile_pool(name="pe_pt", bufs=2))
                pesT = ec(tc.tile_pool(name="pe_sT", bufs=2))
                peo = ec(tc.tile_pool(name="pe_out", bufs=2))
                psE = ec(tc.tile_pool(name="psE", bufs=2, space="PSUM"))
                psH = ec(tc.tile_pool(name="psH", bufs=2, space="PSUM"))
                pdl = ec(tc.tile_pool(name="pd_lhs", bufs=1))
                pdr = ec(tc.tile_pool(name="pd_rhs", bufs=2))
                pdiw = ec(tc.tile_pool(name="pd_iw", bufs=2))
                psD = ec(tc.tile_pool(name="psD", bufs=2, space="PSUM"))
                evictD = ec(tc.tile_pool(name="evDp", bufs=2))
                fuseT_sb = pfc.tile([P, nhc, ic], f16, tag="fuseT_sb")
                nc.gpsimd.dma_start(
                    fuseT_sb[:], fuseT[:, :].rearrange("(k p) o -> p k o", p=P)
                )
                resT_sb = pfc.tile([P, 2 * nic, cout], f16, tag="resT_sb")
                nc.gpsimd.dma_start(
                    resT_sb[:], resT[:, :].rearrange("(k p) o -> p k o", p=P)
                )
                nodes_sb = pec.tile([P, nkn, ic], f16, tag="nodes_sb")
                nc.gpsimd.dma_start(
                    nodes_sb[:], nodes_d[:, :].rearrange("(t p) c -> p t c", p=P)
                )
                hopT_sb = pec.tile([P, hop, nic, ic], f16, tag="hopT_sb")
                for i in range(hop):
                    nc.gpsimd.dma_start(
                        hopT_sb[:, i, :, :],
                        hopT[i, :, :].rearrange("(k p) o -> p k o", p=P),
                    )

                def emit_chain(i, rt, sT, sl, iwt=None):
                    """Softmax chain for hop i, row tile rt; writes the
                    transposed, normalized probabilities into
                    sT[:, :, sl*P:(sl+1)*P]. Hops 0/1 share one iw row
                    load; hop0 uses ACT relu when thr == 0 (b0 o iw ==
                    relu(iw)) so no b0 row load or DVE mul is needed."""
                    if i == 2:
                        lg = pelg.tile([P, n], f32, tag="lg")
                        nc.gpsimd.dma_start(
                            lg[:], lg3d[rt * P:(rt + 1) * P, :]
                        )
                    elif i == 0 and thr == 0.0:
                        lg = pelg.tile([P, n], f32, tag="lg")
                        nc.scalar.activation(lg[:], iwt[:], AF.Relu)
                    else:
                        if i == 0:
                            bht = pebh.tile([P, n], fp8, tag="bh")
                            nc.gpsimd.dma_start(
                                bht[:],
                                b0qb[rt // kpb][
                                    (rt % kpb) * P:(rt % kpb + 1) * P, :
                                ],
                            )
                        else:
                            bht = pebh.tile([P, n], f16, tag="bh")
                            nc.gpsimd.dma_start(
                                bht[:], bh2h[rt * P:(rt + 1) * P, :]
                            )
                        lg = pelg.tile([P, n], f32, tag="lg")
                        nc.vector.tensor_mul(lg[:], iwt[:], bht[:])
                    nmax = None
                    if i > 0:
                        nmax = peo.tile([P, 1], f32, tag="nmax")
                        nc.vector.tensor_reduce(
                            nmax[:], lg[:], axis=AX.X, op=ALU.max, negate=True
                        )
                    zt = peo.tile([P, 1], f32, tag="zt")
                    pt = pept.tile([P, n], f16, tag="pt")
                    if nmax is not None:
                        nc.scalar.activation(
                            pt[:], lg[:], AF.Exp, bias=nmax[:], accum_out=zt[:]
                        )
                    else:
                        nc.scalar.activation(
                            pt[:], lg[:], AF.Exp, accum_out=zt[:]
                        )
                    rz = peo.tile([P, 1], f32, tag="rz")
                    nc.vector.reciprocal(rz[:], zt[:])
                    st = pept.tile([P, n], f16, tag="pt")
                    nc.scalar.activation(st[:], pt[:], AF.Copy, scale=rz[:])
                    nc.scalar.dma_start(
                        sT[:, :, sl * P:(sl + 1) * P], st[:], transpose=True
                    )

                def emit_chains(i, s):
                    """Emit the softmax chains for a todo entry. i == 01
                    emits hops 0 and 1 together, sharing one iw row load
                    per row tile; returns the super descriptors."""
                    if i == 1:
                        sT0 = pesT.tile([P, nkn, SB * P], f16, tag="sT")
                        sT1 = pesT.tile([P, nkn, SB * P], f16, tag="sT")
                        for sl in range(SB):
                            rt = s * SB + sl
                            iwt = peiw.tile([P, n], f32, tag="iwt")
                            nc.gpsimd.dma_start(
                                iwt[:], iwq[rt * P:(rt + 1) * P, :]
                            )
                            emit_chain(0, rt, sT0, sl, iwt)
                            emit_chain(1, rt, sT1, sl, iwt)
                        return [(0, s, sT0), (1, s, sT1)]
                    sT = pesT.tile([P, nkn, SB * P], f16, tag="sT")
                    for sl in range(SB):
                        emit_chain(i, s * SB + sl, sT, sl)
                    return [(i, s, sT)]

                def emit_super_mms(i, s, sT):
                    """t = s @ nodes and the hop conv for super-block s of
                    hop i (SB row tiles at once, FD=SB*P). Evictions stay
                    off ACT so the in-order ACT queue only carries the
                    softmax chains."""
                    W = SB * P
                    tcb = peo.tile([P, nic, W], f16, tag=f"tcb{s % 2}")
                    for c in range(nic):
                        ps = psE.tile([P, W], f32, tag="psE")
                        for j in range(nkn):
                            nc.tensor.matmul(
                                ps[:],
                                nodes_sb[:, j, c * P:(c + 1) * P],
                                sT[:, j, :],
                                start=(j == 0),
                                stop=(j == nkn - 1),
                            )
                        nc.vector.tensor_copy(tcb[:, c, :], ps[:])
                    hv = peo.tile([P, nic, W], f16, tag=f"hv{s % 2}")
                    for o in range(nic):
                        ph = psH.tile([P, W], f32, tag="psH")
                        for c in range(nic):
                            nc.tensor.matmul(
                                ph[:],
                                hopT_sb[:, i, c, o * P:(o + 1) * P],
                                tcb[:, c, :],
                                start=(c == 0),
                                stop=(c == nic - 1),
                            )
                        nc.vector.tensor_scalar(
                            hv[:, o, :],
                            ph[:],
                            bias_sb[
                                :, C_HOP + i * nic + o:C_HOP + i * nic + o + 1
                            ],
                            None,
                            op0=ALU.add,
                        )
                    nc.sync.dma_start(
                        hops_d[i][:, s * W:(s + 1) * W].rearrange(
                            "(o p) q -> p o q", p=P
                        ),
                        hv[:],
                    )

                # Interleave schedule per D block b (nfb=8 slots):
                #   slot m: emit chains for pending hop-super m (if any),
                #           then D's mc=m matmul group,
                #           then consume super m-offset's matmuls.
                # Chains get >=2 D-groups (~50us) of DVE/ACT headroom
                # before their PE matmuls, and every engine queue stays
                # aligned with consumption order.
                for b in range(nrh):
                    lh = pdl.tile([P, nkn, RH], f16, tag="lhD")
                    nc.gpsimd.dma_start(
                        lh[:],
                        bh2T[:, b * RH:(b + 1) * RH].rearrange(
                            "(k p) q -> p k q", p=P
                        ),
                    )
                    todo = []
                    for sp in range(spb):
                        todo.append((1, b * spb + sp))
                    if b > 0:
                        for sp in range(spb):
                            todo.append((2, (b - 1) * spb + sp))
                    nsup = 2 * spb + (spb if b > 0 else 0)
                    offset = nfb - nsup
                    filled = []
                    ncons = 0
                    for mc in range(nfb):
                        if mc >= offset and ncons < len(filled):
                            emit_super_mms(*filled[ncons])
                            ncons += 1
                        rt_ = pdr.tile([P, nkn, FB], fp8, tag="rhsD")
                        for j in range(4):
                            nc.gpsimd.dma_start(
                                rt_[:, j * kpb:(j + 1) * kpb, :],
                                b0qb[j][:, mc * FB:(mc + 1) * FB].rearrange(
                                    "(k p) q -> p k q", p=P
                                ),
                            )
                        for rq in range(nrq):
                            ps = psD.tile(
                                [P, FB], f32, tag=f"psD{rq % 4}", bufs=1
                            )
                            for k in range(nkn):
                                nc.tensor.matmul(
                                    ps[:],
                                    lh[:, k, rq * P:(rq + 1) * P],
                                    rt_[:, k, :],
                                    start=(k == 0),
                                    stop=(k == nkn - 1),
                                )
                            rg = b * RH + rq * P
                            iwd = pdiw.tile(
                                [P, FB], f32, tag=f"iwd{rq % 4}", bufs=1
                            )
                            nc.gpsimd.dma_start(
                                iwd[:],
                                iwq[rg:rg + P, mc * FB:(mc + 1) * FB],
                            )
                            ev = evictD.tile(
                                [P, FB], f32, tag=f"evD{rq % 4}", bufs=1
                            )
                            nc.vector.tensor_mul(ev[:], ps[:], iwd[:])
                            nc.sync.dma_start(
                                lg3d[rg:rg + P, mc * FB:(mc + 1) * FB], ev[:]
                            )
                        if mc < len(todo):
                            filled.extend(emit_chains(*todo[mc]))
                    for k in range(ncons, len(filled)):
                        emit_super_mms(*filled[k])
                # tail: hop 2 of the last D block
                for sp in range(spb):
                    for sup in emit_chains(2, (nrh - 1) * spb + sp):
                        emit_super_mms(*sup)

            # ---------------- Phase F: fuse + residual ----------------
            FR = min(512, r)
            nrf = r // FR
            with (
                tc.tile_pool(name="pf", bufs=1) as pf,
                tc.tile_pool(name="psF", bufs=2, space="PSUM") as psF,
                tc.tile_pool(name="evFp", bufs=2) as evict,
            ):
                hops_sb = pf.tile([P, nhc, r], f16, tag="hops_sb")
                for i in range(hop):
                    nc.gpsimd.dma_start(
                        hops_sb[:, i * nic:(i + 1) * nic, :],
                        hops_d[i][:, :].rearrange("(k p) q -> p k q", p=P),
                    )
                xres_sb = pf.tile([P, nxc, r], f16, tag="xres_sb")
                nc.gpsimd.dma_start(
                    xres_sb[:],
                    x16[0:ic, 0:r].rearrange("(k p) q -> p k q", p=P),
                )
                xp_sb = pf.tile([P, nic, r], f16, tag="xp_sb")

                for o in range(nic):
                    for rf in range(nrf):
                        ps = psF.tile([P, FR], f32, tag="psF")
                        for k in range(nhc):
                            nc.tensor.matmul(
                                ps[:],
                                fuseT_sb[:, k, o * P:(o + 1) * P],
                                hops_sb[:, k, rf * FR:(rf + 1) * FR],
                                start=(k == 0),
                                stop=(k == nhc - 1),
                            )
                        nc.scalar.activation(
                            xp_sb[:, o, rf * FR:(rf + 1) * FR],
                            ps[:],
                            AF.Identity,
                            bias=bias_sb[:, C_FUSE + o:C_FUSE + o + 1],
                        )

                for o in range(ncout):
                    for rf in range(nrf):
                        ps = psF.tile([P, FR], f32, tag="psF")
                        for k in range(nxc):
                            nc.tensor.matmul(
                                ps[:],
                                resT_sb[:, k, o * P:(o + 1) * P],
                                xres_sb[:, k, rf * FR:(rf + 1) * FR],
                                start=(k == 0),
                                stop=False,
                            )
                        for k in range(nic):
                            nc.tensor.matmul(
                                ps[:],
                                resT_sb[:, nxc + k, o * P:(o + 1) * P],
                                xp_sb[:, k, rf * FR:(rf + 1) * FR],
                                start=False,
                                stop=(k == nic - 1),
                            )
                        ev = evict.tile([P, FR], f32, tag="evF")
                        nc.scalar.activation(
                            ev[:],
                            ps[:],
                            AF.Identity,
                            bias=bias_sb[:, C_RES + o:C_RES + o + 1],
                        )
                        nc.sync.dma_start(
                            out[o * P:(o + 1) * P, rf * FR:(rf + 1) * FR], ev[:]
                        )

    nc.compile()
    return nc


def _host_prep(inputs, cin, ic, n, r, hop, eps):
    """Fold BN into weights; build per-core input maps."""

    def f32(a):
        return np.ascontiguousarray(np.asarray(a, dtype=np.float32))

    x = f32(inputs["x"])
    B = x.shape[0]
    xf = x.reshape(B, cin, n)

    s4 = float(ic) ** -0.25
    inv1 = 1.0 / np.sqrt(f32(inputs["bn1_v"]) + eps) * f32(inputs["bn1_g"])
    w1_eff = (inv1[:, None] * f32(inputs["w1_w"])) * s4
    b1_eff = (f32(inputs["w1_b"]) * inv1 + f32(inputs["bn1_b"])
              - f32(inputs["bn1_m"]) * inv1) * s4

    invf = 1.0 / np.sqrt(f32(inputs["bnf_v"]) + eps) * f32(inputs["bnf_g"])
    fuse_eff = invf[:, None] * f32(inputs["fuse_w"])
    fuse_b_eff = (f32(inputs["fuse_b"]) * invf + f32(inputs["bnf_b"])
                  - f32(inputs["bnf_m"]) * invf)

    invr = 1.0 / np.sqrt(f32(inputs["bnr_v"]) + eps) * f32(inputs["bnr_g"])
    res_eff = invr[:, None] * f32(inputs["res_w"])
    res_b_eff = (f32(inputs["res_b"]) * invr + f32(inputs["bnr_b"])
                 - f32(inputs["bnr_m"]) * invr)

    delta = float(np.asarray(inputs["delta"]).reshape(-1)[0])
    if delta <= 0.0:
        thr = -3.0e38
    elif delta >= 1.0:
        thr = 3.0e38
    else:
        thr = float(np.log(delta / (1.0 - delta)))

    nic = ic // P
    ncout = cin // P
    nbias = nic + hop * nic + nic + ncout
    bias_pack = np.zeros((P, nbias), np.float32)
    col = 0
    for oc in range(nic):
        bias_pack[:, col] = b1_eff[oc * P:(oc + 1) * P]
        col += 1
    hop_b = f32(inputs["hop_b"])
    for i in range(hop):
        for oc in range(nic):
            bias_pack[:, col] = hop_b[i, oc * P:(oc + 1) * P]
            col += 1
    for oc in range(nic):
        bias_pack[:, col] = fuse_b_eff[oc * P:(oc + 1) * P]
        col += 1
    for oc in range(ncout):
        bias_pack[:, col] = res_b_eff[oc * P:(oc + 1) * P]
        col += 1

    shared = {
        "w1T": np.ascontiguousarray(w1_eff.T).astype(np.float16),
        "nodeT": np.ascontiguousarray(f32(inputs["node_w"]).T).astype(
            np.float16
        ),
        "nbrow": f32(inputs["node_b"]).reshape(1, ic).astype(np.float16),
        "hopT": np.ascontiguousarray(
            f32(inputs["hop_w"]).transpose(0, 2, 1)
        ).astype(np.float16),
        "fuseT": np.ascontiguousarray(fuse_eff.T).astype(np.float16),
        "resT": np.ascontiguousarray(res_eff.T).astype(np.float16),
        "biases": bias_pack,
    }

    n_cores = (B * n) // r
    halves = n // r
    in_maps = []
    for c in range(n_cores):
        b, h = c // halves, c % halves
        perm = (np.arange(n) + h * r) % n
        m = dict(shared)
        m["x16"] = np.ascontiguousarray(xf[b][:, perm]).astype(np.float16)
        in_maps.append(m)
    return in_maps, thr


_BUILD_CACHE = {}


def kernel(**inputs):
    from concourse import bass_utils

    cin, ic, hop, eps = 512, 256, 3, 1e-5
    x = np.asarray(inputs["x"])
    B, _, H, W = x.shape
    n = H * W
    n_cores = 8
    r = (B * n) // n_cores
    halves = n // r

    in_maps, thr = _host_prep(inputs, cin, ic, n, r, hop, eps)

    key = (cin, ic, n, r, hop, thr)
    if key not in _BUILD_CACHE:
        _BUILD_CACHE[key] = _build(cin, ic, n, r, hop, thr)
    nc = _BUILD_CACHE[key]

    res = bass_utils.run_bass_kernel_spmd(nc, in_maps, core_ids=list(range(n_cores)))

    out = np.empty((B, cin, n), np.float32)
    for c in range(n_cores):
        b, h = c // halves, c % halves
        out[b][:, h * r:(h + 1) * r] = res.results[c]["out"]
    return out.reshape(B, cin, H, W).astype(x.dtype)
